# revision 10
# baseline (speedup 1.0000x reference)
"""Trainium2 Bass kernel for ColumnMixedPrecisionLinear (v3).

Computes out[b,s,o] = bias[o] + sum_i x_i[b,s,:] @ (wq_i * s_i[:,None]).T
where x is [4, 2048, 4096] fp32, wq_i are [4096, 1024] int8 slices of the
weight along the input dim, s_i are per-output-channel scales.

Strategy: data-parallel over tokens across 8 NeuronCores. Each core gets
1024 tokens of x (flattened [8192, 4096]) and the full weights, and computes
its [1024, 4096] output shard (stored transposed, host un-transposes).

v3 vs v2: all transposes moved to the HOST (pure layout prep, like v2's
scale/bias rearrange). No DRAM scratch round trips, no xbar/PE transposes:
  - x uploaded pre-transposed per core: xt [4096d, 1024t] bf16 -> straight
    HWDGE load into SBUF d-major quarter tiles.
  - weights uploaded as one concatenated pre-transposed tensor
    wqt [4096d, 4096o] int8 -> per 512-o-chunk SWDGE cast DMA int8->bf16,
    dequantized in place on DVE by per-slice scale rows (bf16, broadcast
    along free dim).
  - matmul computes the output TRANSPOSED: psum[128o, 512t] =
    wt_blk[128d,128o].T @ xT[128d, 512t], accumulated over 32 d-blocks.
    Each stationary weight block serves 2 moving matmuls (token halves).
  - bias is added during the PSUM->SBUF drain on ACT (per-partition fp32
    bias operand of activation(Identity)) -- no ones-matmul.
Per-core DRAM traffic ~46 MB (v2: ~134 MB); PE does only the 2048 main
matmuls (v2: 2112).

int8 weights are exact in bf16; x rounds once to bf16, scales round to
bf16, w*s product rounds to bf16; output rel err ~3e-3.
"""

import numpy as np
import ml_dtypes

import concourse.bass as bass
import concourse.mybir as mybir
import concourse.tile as tile
from concourse import bacc
from concourse.bass_utils import run_bass_kernel_spmd

P = 128
N_CORES = 8
B, S = 4, 2048
D_IN_SLICE = 1024
N_SLICES = 4
D = D_IN_SLICE * N_SLICES      # 4096 contraction dim
O = 4096                       # out features
T = (B * S) // N_CORES         # 1024 tokens per core

D_BLKS = D // P                # 32
D_BLKS_SLICE = D_IN_SLICE // P # 8
O_CHUNK = 512
O_CHUNKS = O // O_CHUNK        # 8
O_TILES_PER_CHUNK = O_CHUNK // P  # 4
T_HALF = T // 2                # 512 moving tokens per matmul
XQ = 8                         # x loaded in 8 eighth tiles (4 d-blocks each)

BF16 = mybir.dt.bfloat16
FP32 = mybir.dt.float32
INT8 = mybir.dt.int8


def build_nc():
    nc = bacc.Bacc(None, target_bir_lowering=False)

    # host-pretransposed inputs
    xt_in = nc.dram_tensor("xt", [D, T], BF16, kind="ExternalInput")
    wqt_in = nc.dram_tensor("wqt", [D, O], INT8, kind="ExternalInput")
    # scb[i][p, o] = bf16(s_i[o]) replicated across partitions
    scb_in = [
        nc.dram_tensor(f"scb{i}", [P, O], BF16, kind="ExternalInput")
        for i in range(N_SLICES)
    ]
    # biasc[p, G] = bias[G*128 + p] for global o-tile G
    biasc_in = nc.dram_tensor("biasc", [P, O // P], FP32, kind="ExternalInput")
    # output stored transposed in bf16 (host casts back; ~2e-3 extra rounding)
    outT = nc.dram_tensor("outT", [O, T], BF16, kind="ExternalOutput")

    with tile.TileContext(nc) as tc:
        with (
            tc.tile_pool(name="const", bufs=1) as const,
            tc.tile_pool(name="xres", bufs=1) as xres,
            tc.tile_pool(name="wt", bufs=2) as wt_pool,
            tc.tile_pool(name="ostage", bufs=2) as ostage,
            tc.tile_pool(name="psm", bufs=1, space="PSUM") as psm,
        ):
            # scales/bias ride the ACT HWDGE queue so the gpsimd queue is
            # free for the critical first weight chunk
            scbs = []
            for i in range(N_SLICES):
                sct = const.tile([P, O], BF16, tag=f"scb{i}")
                if i == 0:
                    # split so chunk 0's first dequant is gated by a small DMA
                    nc.scalar.dma_start(sct[:, 0:O_CHUNK],
                                        scb_in[i][:, 0:O_CHUNK])
                    nc.scalar.dma_start(sct[:, O_CHUNK:],
                                        scb_in[i][:, O_CHUNK:])
                else:
                    nc.scalar.dma_start(sct[:], scb_in[i][:])
                scbs.append(sct)
            biasc = const.tile([P, O // P], FP32)
            nc.scalar.dma_start(biasc[:], biasc_in[:])

            # x: [4096, 1024] bf16 -> 4 quarter tiles [128, 8, 1024],
            # d = (q*8 + db)*128 + p
            xTq = []
            for q in range(XQ):
                xq = xres.tile([P, D_BLKS // XQ, T], BF16, tag=f"xTq{q}",
                               name=f"xTq{q}")
                nc.sync.dma_start(
                    xq[:],
                    xt_in[q * (D // XQ):(q + 1) * (D // XQ), :]
                    .rearrange("(db p) t -> p db t", p=P),
                )
                xTq.append(xq)

            def load_wt_chunk(c, split_first=False):
                # weight chunk: [4096d, 512o] int8 -> [128, 32db, 512] bf16,
                # one cast-DMA + dequant per slice so slice 0 is ready early.
                # split_first halves slice 0's load/dequant (chunk 0 startup).
                wt = wt_pool.tile([P, D_BLKS, O_CHUNK], BF16, tag="wt",
                                  name="wt")
                for i in range(N_SLICES):
                    parts = 2 if (split_first and i == 0) else 1
                    nb = D_BLKS_SLICE // parts
                    for k in range(parts):
                        sl = slice(i * D_BLKS_SLICE + k * nb,
                                   i * D_BLKS_SLICE + (k + 1) * nb)
                        d0 = i * D_IN_SLICE + k * nb * P
                        nc.gpsimd.dma_start(
                            wt[:, sl, :],
                            wqt_in[d0:d0 + nb * P,
                                   c * O_CHUNK:(c + 1) * O_CHUNK]
                            .rearrange("(db p) o -> p db o", p=P),
                        )
                        nc.vector.tensor_tensor(
                            wt[:, sl, :], wt[:, sl, :],
                            scbs[i][:, None, c * O_CHUNK:(c + 1) * O_CHUNK]
                            .to_broadcast((P, nb, O_CHUNK)),
                            mybir.AluOpType.mult,
                        )
                return wt

            def drain(ob, ps, c, g, h):
                # PSUM -> SBUF bf16 with fp32 bias add on ACT
                G = c * O_TILES_PER_CHUNK + g
                nc.scalar.activation(
                    ob[:, g, h * T_HALF:(h + 1) * T_HALF],
                    ps[:],
                    mybir.ActivationFunctionType.Identity,
                    bias=biasc[:, G:G + 1],
                    scale=1.0,
                )

            def store_g(ob, c, g):
                G = c * O_TILES_PER_CHUNK + g
                nc.sync.dma_start(outT[G * P:(G + 1) * P, :], ob[:, g, :])

            def mk_ps(g, h):
                k = g * 2 + h
                return psm.tile([P, T_HALF], FP32, tag=f"ps{k}",
                                name=f"ps{k}")

            def rhs(db, h):
                return xTq[db // D_BLKS_SLICE][
                    :, db % D_BLKS_SLICE, h * T_HALF:(h + 1) * T_HALF]

            # ---- chunk 0: db-outer over all 8 psum banks, so the PE
            # consumes x quarter tiles as they arrive ----
            wt = load_wt_chunk(0, split_first=True)
            ob = ostage.tile([P, O_TILES_PER_CHUNK, T], BF16, tag="ob",
                             name="ob")
            ps_all = [[mk_ps(g, h) for h in range(2)]
                      for g in range(O_TILES_PER_CHUNK)]
            for db in range(D_BLKS):
                for g in range(O_TILES_PER_CHUNK):
                    lhsT = wt[:, db, g * P:(g + 1) * P]
                    for h in range(2):
                        nc.tensor.matmul(
                            ps_all[g][h][:], lhsT, rhs(db, h),
                            start=(db == 0), stop=(db == D_BLKS - 1),
                        )
            for g in range(O_TILES_PER_CHUNK):
                for h in range(2):
                    drain(ob, ps_all[g][h], 0, g, h)
                store_g(ob, 0, g)

            # ---- chunks 1..7: g-outer (staggered drains) ----
            for c in range(1, O_CHUNKS):
                wt = load_wt_chunk(c)
                ob = ostage.tile([P, O_TILES_PER_CHUNK, T], BF16, tag="ob",
                                 name="ob")
                for g in range(O_TILES_PER_CHUNK):
                    ps = [mk_ps(g, h) for h in range(2)]
                    for db in range(D_BLKS):
                        lhsT = wt[:, db, g * P:(g + 1) * P]
                        for h in range(2):
                            nc.tensor.matmul(
                                ps[h][:], lhsT, rhs(db, h),
                                start=(db == 0), stop=(db == D_BLKS - 1),
                            )
                    for h in range(2):
                        drain(ob, ps[h], c, g, h)
                    store_g(ob, c, g)
    nc.compile()
    return nc


_NC_CACHE = None


def _get_nc():
    global _NC_CACHE
    if _NC_CACHE is None:
        _NC_CACHE = build_nc()
    return _NC_CACHE


def _prep_inputs(x, wqs, ss, bias):
    xb = np.asarray(x, dtype=np.float32).reshape(B * S, D).astype(
        ml_dtypes.bfloat16)
    wqt = np.ascontiguousarray(
        np.concatenate(
            [np.asarray(w).astype(np.int8).T for w in wqs], axis=0))
    scbs = [
        np.ascontiguousarray(
            np.broadcast_to(
                np.asarray(s, dtype=np.float32).astype(ml_dtypes.bfloat16),
                (P, O)))
        for s in ss
    ]
    biasc = np.ascontiguousarray(
        np.asarray(bias, dtype=np.float32).reshape(O // P, P).T)
    in_maps = []
    for c in range(N_CORES):
        m = {
            "xt": np.ascontiguousarray(xb[c * T:(c + 1) * T, :].T),
            "wqt": wqt,
            "biasc": biasc,
        }
        for i in range(N_SLICES):
            m[f"scb{i}"] = scbs[i]
        in_maps.append(m)
    return in_maps


def run_on_hw(x, wqs, ss, bias, **spmd_kwargs):
    """Run and return (out_full [B,S,O] fp32, BassKernelResults)."""
    nc = _get_nc()
    in_maps = _prep_inputs(x, wqs, ss, bias)
    res = run_bass_kernel_spmd(nc, in_maps, core_ids=list(range(N_CORES)),
                               **spmd_kwargs)
    # each core returns outT [O, T]; tokens concatenate along axis 1
    out = np.concatenate([r["outT"] for r in res.results], axis=1)
    return np.ascontiguousarray(out.T.reshape(B, S, O).astype(np.float32)), res


def kernel(x, wq0, s0, wq1, s1, wq2, s2, wq3, s3, bias):
    out, _ = run_on_hw(x, [wq0, wq1, wq2, wq3], [s0, s1, s2, s3], bias)
    return out


# revision 12
# speedup vs baseline: 1.6167x; 1.6167x over previous
"""Trainium2 Bass kernel for ColumnMixedPrecisionLinear (v3).

Computes out[b,s,o] = bias[o] + sum_i x_i[b,s,:] @ (wq_i * s_i[:,None]).T
where x is [4, 2048, 4096] fp32, wq_i are [4096, 1024] int8 slices of the
weight along the input dim, s_i are per-output-channel scales.

Strategy: data-parallel over tokens across 8 NeuronCores. Each core gets
1024 tokens of x (flattened [8192, 4096]) and the full weights, and computes
its [1024, 4096] output shard (stored transposed, host un-transposes).

v3 vs v2: all transposes moved to the HOST (pure layout prep, like v2's
scale/bias rearrange). No DRAM scratch round trips, no xbar/PE transposes:
  - x uploaded pre-transposed per core: xt [4096d, 1024t] bf16 -> straight
    HWDGE load into SBUF d-major quarter tiles.
  - weights uploaded as one concatenated pre-transposed tensor
    wqt [4096d, 4096o] int8 -> per 512-o-chunk SWDGE cast DMA int8->bf16,
    dequantized in place on DVE by per-slice scale rows (bf16, broadcast
    along free dim).
  - matmul computes the output TRANSPOSED: psum[128o, 512t] =
    wt_blk[128d,128o].T @ xT[128d, 512t], accumulated over 32 d-blocks.
    Each stationary weight block serves 2 moving matmuls (token halves).
  - bias is added during the PSUM->SBUF drain on ACT (per-partition fp32
    bias operand of activation(Identity)) -- no ones-matmul.
Per-core DRAM traffic ~46 MB (v2: ~134 MB); PE does only the 2048 main
matmuls (v2: 2112).

int8 weights are exact in bf16; x rounds once to bf16, scales round to
bf16, w*s product rounds to bf16; output rel err ~3e-3.
"""

import numpy as np
import ml_dtypes

import concourse.bass as bass
import concourse.mybir as mybir
import concourse.tile as tile
from concourse import bacc
from concourse.bass_utils import run_bass_kernel_spmd

P = 128
N_CORES = 8
B, S = 4, 2048
D_IN_SLICE = 1024
N_SLICES = 4
D = D_IN_SLICE * N_SLICES      # 4096 contraction dim
O = 4096                       # out features
T = (B * S) // N_CORES         # 1024 tokens per core

D_BLKS = D // P                # 32
D_BLKS_SLICE = D_IN_SLICE // P # 8
O_CHUNK = 512
O_CHUNKS = O // O_CHUNK        # 8
O_TILES_PER_CHUNK = O_CHUNK // P  # 4
T_HALF = T // 2                # 512 moving tokens per matmul
XQ = 8                         # x loaded in 8 eighth tiles (4 d-blocks each)

BF16 = mybir.dt.bfloat16
FP32 = mybir.dt.float32
INT8 = mybir.dt.int8


def build_nc():
    nc = bacc.Bacc(None, target_bir_lowering=False)

    # host-pretransposed inputs
    xt_in = nc.dram_tensor("xt", [D, T], BF16, kind="ExternalInput")
    wqt_in = nc.dram_tensor("wqt", [D, O], INT8, kind="ExternalInput")
    # scb[i][p, o] = bf16(s_i[o]) replicated across partitions
    scb_in = [
        nc.dram_tensor(f"scb{i}", [P, O], BF16, kind="ExternalInput")
        for i in range(N_SLICES)
    ]
    # biasc[p, G] = bias[G*128 + p] for global o-tile G
    biasc_in = nc.dram_tensor("biasc", [P, O // P], FP32, kind="ExternalInput")
    # output stored transposed in bf16 (host casts back; ~2e-3 extra rounding)
    outT = nc.dram_tensor("outT", [O, T], BF16, kind="ExternalOutput")

    with tile.TileContext(nc) as tc:
        with (
            tc.tile_pool(name="const", bufs=1) as const,
            tc.tile_pool(name="xres", bufs=1) as xres,
            tc.tile_pool(name="wt", bufs=2) as wt_pool,
            tc.tile_pool(name="ostage", bufs=2) as ostage,
            tc.tile_pool(name="psm", bufs=1, space="PSUM") as psm,
        ):
            # scales/bias ride the ACT HWDGE queue so the gpsimd queue is
            # free for the critical first weight chunk
            scbs = []
            for i in range(N_SLICES):
                sct = const.tile([P, O], BF16, tag=f"scb{i}")
                if i == 0:
                    # split so chunk 0's first dequant is gated by a small DMA
                    nc.scalar.dma_start(sct[:, 0:O_CHUNK],
                                        scb_in[i][:, 0:O_CHUNK])
                    nc.scalar.dma_start(sct[:, O_CHUNK:],
                                        scb_in[i][:, O_CHUNK:])
                else:
                    nc.scalar.dma_start(sct[:], scb_in[i][:])
                scbs.append(sct)
            biasc = const.tile([P, O // P], FP32)
            nc.scalar.dma_start(biasc[:], biasc_in[:])

            # x: [4096, 1024] bf16 -> 4 quarter tiles [128, 8, 1024],
            # d = (q*8 + db)*128 + p
            xTq = []
            for q in range(XQ):
                xq = xres.tile([P, D_BLKS // XQ, T], BF16, tag=f"xTq{q}",
                               name=f"xTq{q}")
                nc.sync.dma_start(
                    xq[:],
                    xt_in[q * (D // XQ):(q + 1) * (D // XQ), :]
                    .rearrange("(db p) t -> p db t", p=P),
                )
                xTq.append(xq)

            def load_wt_chunk(c, split_first=False):
                # weight chunk: [4096d, 512o] int8 -> [128, 32db, 512] bf16,
                # one cast-DMA + dequant per slice so slice 0 is ready early.
                # split_first halves slice 0's load/dequant (chunk 0 startup).
                wt = wt_pool.tile([P, D_BLKS, O_CHUNK], BF16, tag="wt",
                                  name="wt")
                for i in range(N_SLICES):
                    parts = 2 if (split_first and i == 0) else 1
                    nb = D_BLKS_SLICE // parts
                    for k in range(parts):
                        sl = slice(i * D_BLKS_SLICE + k * nb,
                                   i * D_BLKS_SLICE + (k + 1) * nb)
                        d0 = i * D_IN_SLICE + k * nb * P
                        nc.gpsimd.dma_start(
                            wt[:, sl, :],
                            wqt_in[d0:d0 + nb * P,
                                   c * O_CHUNK:(c + 1) * O_CHUNK]
                            .rearrange("(db p) o -> p db o", p=P),
                        )
                        nc.vector.tensor_tensor(
                            wt[:, sl, :], wt[:, sl, :],
                            scbs[i][:, None, c * O_CHUNK:(c + 1) * O_CHUNK]
                            .to_broadcast((P, nb, O_CHUNK)),
                            mybir.AluOpType.mult,
                        )
                return wt

            def drain(ob, ps, c, g, h):
                # PSUM -> SBUF bf16 with fp32 bias add on ACT
                G = c * O_TILES_PER_CHUNK + g
                nc.scalar.activation(
                    ob[:, g, h * T_HALF:(h + 1) * T_HALF],
                    ps[:],
                    mybir.ActivationFunctionType.Identity,
                    bias=biasc[:, G:G + 1],
                    scale=1.0,
                )

            def store_g(ob, c, g):
                G = c * O_TILES_PER_CHUNK + g
                nc.sync.dma_start(outT[G * P:(G + 1) * P, :], ob[:, g, :])

            def mk_ps(g, h):
                k = g * 2 + h
                return psm.tile([P, T_HALF], FP32, tag=f"ps{k}",
                                name=f"ps{k}")

            XB = D_BLKS // XQ      # d-blocks per x tile

            def rhs(db, h):
                return xTq[db // XB][:, db % XB,
                                     h * T_HALF:(h + 1) * T_HALF]

            # ---- chunk 0: db-outer over all 8 psum banks, so the PE
            # consumes x quarter tiles as they arrive ----
            wt = load_wt_chunk(0, split_first=True)
            ob = ostage.tile([P, O_TILES_PER_CHUNK, T], BF16, tag="ob",
                             name="ob")
            ps_all = [[mk_ps(g, h) for h in range(2)]
                      for g in range(O_TILES_PER_CHUNK)]
            for db in range(D_BLKS):
                for g in range(O_TILES_PER_CHUNK):
                    lhsT = wt[:, db, g * P:(g + 1) * P]
                    for h in range(2):
                        nc.tensor.matmul(
                            ps_all[g][h][:], lhsT, rhs(db, h),
                            start=(db == 0), stop=(db == D_BLKS - 1),
                        )
            for g in range(O_TILES_PER_CHUNK):
                for h in range(2):
                    drain(ob, ps_all[g][h], 0, g, h)
                store_g(ob, 0, g)

            # ---- chunks 1..7: g-outer (staggered drains) ----
            for c in range(1, O_CHUNKS):
                wt = load_wt_chunk(c)
                ob = ostage.tile([P, O_TILES_PER_CHUNK, T], BF16, tag="ob",
                                 name="ob")
                for g in range(O_TILES_PER_CHUNK):
                    ps = [mk_ps(g, h) for h in range(2)]
                    for db in range(D_BLKS):
                        lhsT = wt[:, db, g * P:(g + 1) * P]
                        for h in range(2):
                            nc.tensor.matmul(
                                ps[h][:], lhsT, rhs(db, h),
                                start=(db == 0), stop=(db == D_BLKS - 1),
                            )
                    for h in range(2):
                        drain(ob, ps[h], c, g, h)
                    if c == O_CHUNKS - 1 and g == O_TILES_PER_CHUNK - 1:
                        # overlap the final drains/stores on two HWDGE queues
                        G = c * O_TILES_PER_CHUNK + g
                        nc.sync.dma_start(
                            outT[G * P:(G + 1) * P, 0:T_HALF],
                            ob[:, g, 0:T_HALF])
                        nc.scalar.dma_start(
                            outT[G * P:(G + 1) * P, T_HALF:],
                            ob[:, g, T_HALF:])
                    else:
                        store_g(ob, c, g)
    nc.compile()
    return nc


_NC_CACHE = None


def _get_nc():
    global _NC_CACHE
    if _NC_CACHE is None:
        _NC_CACHE = build_nc()
    return _NC_CACHE


def _prep_inputs(x, wqs, ss, bias):
    xb = np.asarray(x, dtype=np.float32).reshape(B * S, D).astype(
        ml_dtypes.bfloat16)
    wqt = np.ascontiguousarray(
        np.concatenate(
            [np.asarray(w).astype(np.int8).T for w in wqs], axis=0))
    scbs = [
        np.ascontiguousarray(
            np.broadcast_to(
                np.asarray(s, dtype=np.float32).astype(ml_dtypes.bfloat16),
                (P, O)))
        for s in ss
    ]
    biasc = np.ascontiguousarray(
        np.asarray(bias, dtype=np.float32).reshape(O // P, P).T)
    in_maps = []
    for c in range(N_CORES):
        m = {
            "xt": np.ascontiguousarray(xb[c * T:(c + 1) * T, :].T),
            "wqt": wqt,
            "biasc": biasc,
        }
        for i in range(N_SLICES):
            m[f"scb{i}"] = scbs[i]
        in_maps.append(m)
    return in_maps


def run_on_hw(x, wqs, ss, bias, **spmd_kwargs):
    """Run and return (out_full [B,S,O] fp32, BassKernelResults)."""
    nc = _get_nc()
    in_maps = _prep_inputs(x, wqs, ss, bias)
    res = run_bass_kernel_spmd(nc, in_maps, core_ids=list(range(N_CORES)),
                               **spmd_kwargs)
    # each core returns outT [O, T]; tokens concatenate along axis 1
    out = np.concatenate([r["outT"] for r in res.results], axis=1)
    return np.ascontiguousarray(out.T.reshape(B, S, O).astype(np.float32)), res


def kernel(x, wq0, s0, wq1, s1, wq2, s2, wq3, s3, bias):
    out, _ = run_on_hw(x, [wq0, wq1, wq2, wq3], [s0, s1, s2, s3], bias)
    return out


# revision 15
# speedup vs baseline: 1.8740x; 1.1592x over previous
"""Trainium2 Bass kernel for ColumnMixedPrecisionLinear (v3).

Computes out[b,s,o] = bias[o] + sum_i x_i[b,s,:] @ (wq_i * s_i[:,None]).T
where x is [4, 2048, 4096] fp32, wq_i are [4096, 1024] int8 slices of the
weight along the input dim, s_i are per-output-channel scales.

Strategy: data-parallel over tokens across 8 NeuronCores. Each core gets
1024 tokens of x (flattened [8192, 4096]) and the full weights, and computes
its [1024, 4096] output shard (stored transposed, host un-transposes).

v3 vs v2: all transposes moved to the HOST (pure layout prep, like v2's
scale/bias rearrange). No DRAM scratch round trips, no xbar/PE transposes:
  - x uploaded pre-transposed per core: xt [4096d, 1024t] bf16 -> straight
    HWDGE load into SBUF d-major quarter tiles.
  - weights uploaded as one concatenated pre-transposed tensor
    wqt [4096d, 4096o] int8 -> per 512-o-chunk SWDGE cast DMA int8->bf16,
    dequantized in place on DVE by per-slice scale rows (bf16, broadcast
    along free dim).
  - matmul computes the output TRANSPOSED: psum[128o, 512t] =
    wt_blk[128d,128o].T @ xT[128d, 512t], accumulated over 32 d-blocks.
    Each stationary weight block serves 2 moving matmuls (token halves).
  - bias is added during the PSUM->SBUF drain on ACT (per-partition fp32
    bias operand of activation(Identity)) -- no ones-matmul.
Per-core DRAM traffic ~46 MB (v2: ~134 MB); PE does only the 2048 main
matmuls (v2: 2112).

int8 weights are exact in bf16; x rounds once to bf16, scales round to
bf16, w*s product rounds to bf16; output rel err ~3e-3.
"""

import numpy as np
import ml_dtypes

import concourse.bass as bass
import concourse.mybir as mybir
import concourse.tile as tile
from concourse import bacc
from concourse.bass_utils import run_bass_kernel_spmd

P = 128
N_CORES = 8
B, S = 4, 2048
D_IN_SLICE = 1024
N_SLICES = 4
D = D_IN_SLICE * N_SLICES      # 4096 contraction dim
O = 4096                       # out features
T = (B * S) // N_CORES         # 1024 tokens per core

D_BLKS = D // P                # 32
D_BLKS_SLICE = D_IN_SLICE // P # 8
O_CHUNK = 512
O_CHUNKS = O // O_CHUNK        # 8
O_TILES_PER_CHUNK = O_CHUNK // P  # 4
T_HALF = T // 2                # 512 moving tokens per matmul
XQ = 16                        # x loaded in 16 tiles (2 d-blocks each)

BF16 = mybir.dt.bfloat16
FP32 = mybir.dt.float32
INT8 = mybir.dt.int8


def build_nc():
    nc = bacc.Bacc(None, target_bir_lowering=False)

    # host-pretransposed inputs
    xt_in = nc.dram_tensor("xt", [D, T], BF16, kind="ExternalInput")
    wqt_in = nc.dram_tensor("wqt", [D, O], INT8, kind="ExternalInput")
    # scb[i][p, o] = bf16(s_i[o]) replicated across partitions
    scb_in = [
        nc.dram_tensor(f"scb{i}", [P, O], BF16, kind="ExternalInput")
        for i in range(N_SLICES)
    ]
    # biasc[p, G] = bias[G*128 + p] for global o-tile G
    biasc_in = nc.dram_tensor("biasc", [P, O // P], FP32, kind="ExternalInput")
    # output stored transposed in bf16 (host casts back; ~2e-3 extra rounding)
    outT = nc.dram_tensor("outT", [O, T], BF16, kind="ExternalOutput")

    with tile.TileContext(nc) as tc:
        with (
            tc.tile_pool(name="const", bufs=1) as const,
            tc.tile_pool(name="xres", bufs=1) as xres,
            tc.tile_pool(name="wt", bufs=2) as wt_pool,
            tc.tile_pool(name="ostage", bufs=2) as ostage,
            tc.tile_pool(name="psm", bufs=1, space="PSUM") as psm,
        ):
            # scales/bias ride the ACT HWDGE queue so the gpsimd queue is
            # free for the critical first weight chunk
            scbs = []
            for i in range(N_SLICES):
                sct = const.tile([P, O], BF16, tag=f"scb{i}")
                if i == 0:
                    # split so chunk 0's first dequant is gated by a small DMA
                    nc.scalar.dma_start(sct[:, 0:O_CHUNK],
                                        scb_in[i][:, 0:O_CHUNK])
                    nc.scalar.dma_start(sct[:, O_CHUNK:],
                                        scb_in[i][:, O_CHUNK:])
                else:
                    nc.scalar.dma_start(sct[:], scb_in[i][:])
                scbs.append(sct)
            biasc = const.tile([P, O // P], FP32)
            nc.scalar.dma_start(biasc[:], biasc_in[:])

            # x: [4096, 1024] bf16 -> XQ tiles [128, 32/XQ, 1024],
            # d = (q*(32/XQ) + db)*128 + p
            xTq = []
            for q in range(XQ):
                xq = xres.tile([P, D_BLKS // XQ, T], BF16, tag=f"xTq{q}",
                               name=f"xTq{q}")
                nc.sync.dma_start(
                    xq[:],
                    xt_in[q * (D // XQ):(q + 1) * (D // XQ), :]
                    .rearrange("(db p) t -> p db t", p=P),
                )
                xTq.append(xq)

            def load_wt_chunk(c, split_first=False):
                # weight chunk: [4096d, 512o] int8 -> [128, 32db, 512] bf16,
                # one cast-DMA + dequant per slice so slice 0 is ready early.
                # split_first halves slice 0's load/dequant (chunk 0 startup).
                wt = wt_pool.tile([P, D_BLKS, O_CHUNK], BF16, tag="wt",
                                  name="wt")
                for i in range(N_SLICES):
                    parts = 2 if (split_first and i == 0) else 1
                    nb = D_BLKS_SLICE // parts
                    for k in range(parts):
                        sl = slice(i * D_BLKS_SLICE + k * nb,
                                   i * D_BLKS_SLICE + (k + 1) * nb)
                        d0 = i * D_IN_SLICE + k * nb * P
                        nc.gpsimd.dma_start(
                            wt[:, sl, :],
                            wqt_in[d0:d0 + nb * P,
                                   c * O_CHUNK:(c + 1) * O_CHUNK]
                            .rearrange("(db p) o -> p db o", p=P),
                        )
                        nc.vector.tensor_tensor(
                            wt[:, sl, :], wt[:, sl, :],
                            scbs[i][:, None, c * O_CHUNK:(c + 1) * O_CHUNK]
                            .to_broadcast((P, nb, O_CHUNK)),
                            mybir.AluOpType.mult,
                        )
                return wt

            def drain(ob, ps, c, g, h):
                # PSUM -> SBUF bf16 with fp32 bias add on ACT
                G = c * O_TILES_PER_CHUNK + g
                nc.scalar.activation(
                    ob[:, g, h * T_HALF:(h + 1) * T_HALF],
                    ps[:],
                    mybir.ActivationFunctionType.Identity,
                    bias=biasc[:, G:G + 1],
                    scale=1.0,
                )

            def store_g(ob, c, g):
                G = c * O_TILES_PER_CHUNK + g
                nc.sync.dma_start(outT[G * P:(G + 1) * P, :], ob[:, g, :])

            def mk_ps(g, h):
                k = g * 2 + h
                return psm.tile([P, T_HALF], FP32, tag=f"ps{k}",
                                name=f"ps{k}")

            XB = D_BLKS // XQ      # d-blocks per x tile

            def rhs(db, h):
                return xTq[db // XB][:, db % XB,
                                     h * T_HALF:(h + 1) * T_HALF]

            # ---- chunk 0: db-outer over all 8 psum banks, so the PE
            # consumes x quarter tiles as they arrive ----
            wt = load_wt_chunk(0, split_first=True)
            ob = ostage.tile([P, O_TILES_PER_CHUNK, T], BF16, tag="ob",
                             name="ob")
            ps_all = [[mk_ps(g, h) for h in range(2)]
                      for g in range(O_TILES_PER_CHUNK)]
            for db in range(D_BLKS):
                for g in range(O_TILES_PER_CHUNK):
                    lhsT = wt[:, db, g * P:(g + 1) * P]
                    for h in range(2):
                        nc.tensor.matmul(
                            ps_all[g][h][:], lhsT, rhs(db, h),
                            start=(db == 0), stop=(db == D_BLKS - 1),
                        )
            for g in range(O_TILES_PER_CHUNK):
                for h in range(2):
                    drain(ob, ps_all[g][h], 0, g, h)
                store_g(ob, 0, g)

            # ---- chunks 1..7: g-outer (staggered drains) ----
            for c in range(1, O_CHUNKS):
                wt = load_wt_chunk(c)
                ob = ostage.tile([P, O_TILES_PER_CHUNK, T], BF16, tag="ob",
                                 name="ob")
                for g in range(O_TILES_PER_CHUNK):
                    ps = [mk_ps(g, h) for h in range(2)]
                    for db in range(D_BLKS):
                        lhsT = wt[:, db, g * P:(g + 1) * P]
                        for h in range(2):
                            nc.tensor.matmul(
                                ps[h][:], lhsT, rhs(db, h),
                                start=(db == 0), stop=(db == D_BLKS - 1),
                            )
                    for h in range(2):
                        drain(ob, ps[h], c, g, h)
                    if c == O_CHUNKS - 1 and g == O_TILES_PER_CHUNK - 1:
                        # overlap the final drains/stores on two HWDGE queues
                        G = c * O_TILES_PER_CHUNK + g
                        nc.sync.dma_start(
                            outT[G * P:(G + 1) * P, 0:T_HALF],
                            ob[:, g, 0:T_HALF])
                        nc.scalar.dma_start(
                            outT[G * P:(G + 1) * P, T_HALF:],
                            ob[:, g, T_HALF:])
                    else:
                        store_g(ob, c, g)
    nc.compile()
    return nc


_NC_CACHE = None


def _get_nc():
    global _NC_CACHE
    if _NC_CACHE is None:
        _NC_CACHE = build_nc()
    return _NC_CACHE


def _prep_inputs(x, wqs, ss, bias):
    xb = np.asarray(x, dtype=np.float32).reshape(B * S, D).astype(
        ml_dtypes.bfloat16)
    wqt = np.ascontiguousarray(
        np.concatenate(
            [np.asarray(w).astype(np.int8).T for w in wqs], axis=0))
    scbs = [
        np.ascontiguousarray(
            np.broadcast_to(
                np.asarray(s, dtype=np.float32).astype(ml_dtypes.bfloat16),
                (P, O)))
        for s in ss
    ]
    biasc = np.ascontiguousarray(
        np.asarray(bias, dtype=np.float32).reshape(O // P, P).T)
    in_maps = []
    for c in range(N_CORES):
        m = {
            "xt": np.ascontiguousarray(xb[c * T:(c + 1) * T, :].T),
            "wqt": wqt,
            "biasc": biasc,
        }
        for i in range(N_SLICES):
            m[f"scb{i}"] = scbs[i]
        in_maps.append(m)
    return in_maps


def run_on_hw(x, wqs, ss, bias, **spmd_kwargs):
    """Run and return (out_full [B,S,O] fp32, BassKernelResults)."""
    nc = _get_nc()
    in_maps = _prep_inputs(x, wqs, ss, bias)
    res = run_bass_kernel_spmd(nc, in_maps, core_ids=list(range(N_CORES)),
                               **spmd_kwargs)
    # each core returns outT [O, T]; tokens concatenate along axis 1
    out = np.concatenate([r["outT"] for r in res.results], axis=1)
    return np.ascontiguousarray(out.T.reshape(B, S, O).astype(np.float32)), res


def kernel(x, wq0, s0, wq1, s1, wq2, s2, wq3, s3, bias):
    out, _ = run_on_hw(x, [wq0, wq1, wq2, wq3], [s0, s1, s2, s3], bias)
    return out


# revision 20
# speedup vs baseline: 2.5490x; 1.3602x over previous
"""Trainium2 Bass kernel for ColumnMixedPrecisionLinear (v3).

Computes out[b,s,o] = bias[o] + sum_i x_i[b,s,:] @ (wq_i * s_i[:,None]).T
where x is [4, 2048, 4096] fp32, wq_i are [4096, 1024] int8 slices of the
weight along the input dim, s_i are per-output-channel scales.

Strategy: data-parallel over tokens across 8 NeuronCores. Each core gets
1024 tokens of x (flattened [8192, 4096]) and the full weights, and computes
its [1024, 4096] output shard (stored transposed, host un-transposes).

v3 vs v2: all transposes moved to the HOST (pure layout prep, like v2's
scale/bias rearrange). No DRAM scratch round trips, no xbar/PE transposes:
  - x uploaded pre-transposed per core: xt [4096d, 1024t] bf16 -> straight
    HWDGE load into SBUF d-major quarter tiles.
  - weights uploaded as one concatenated pre-transposed tensor
    wqt [4096d, 4096o] int8 -> per 512-o-chunk SWDGE cast DMA int8->bf16,
    dequantized in place on DVE by per-slice scale rows (bf16, broadcast
    along free dim).
  - matmul computes the output TRANSPOSED: psum[128o, 512t] =
    wt_blk[128d,128o].T @ xT[128d, 512t], accumulated over 32 d-blocks.
    Each stationary weight block serves 2 moving matmuls (token halves).
  - bias is added during the PSUM->SBUF drain on ACT (per-partition fp32
    bias operand of activation(Identity)) -- no ones-matmul.
Per-core DRAM traffic ~46 MB (v2: ~134 MB); PE does only the 2048 main
matmuls (v2: 2112).

int8 weights are exact in bf16; x rounds once to bf16, scales round to
bf16, w*s product rounds to bf16; output rel err ~3e-3.
"""

import numpy as np
import ml_dtypes

import concourse.bass as bass
import concourse.mybir as mybir
import concourse.tile as tile
from concourse import bacc
from concourse.bass_utils import run_bass_kernel_spmd

P = 128
N_CORES = 8
B, S = 4, 2048
D_IN_SLICE = 1024
N_SLICES = 4
D = D_IN_SLICE * N_SLICES      # 4096 contraction dim
O = 4096                       # out features
T = (B * S) // N_CORES         # 1024 tokens per core

D_BLKS = D // P                # 32
D_BLKS_SLICE = D_IN_SLICE // P # 8
O_CHUNK = 512
O_CHUNKS = O // O_CHUNK        # 8
O_TILES_PER_CHUNK = O_CHUNK // P  # 4
T_HALF = T // 2                # 512 moving tokens per matmul
XQ = 16                        # x loaded in 16 tiles (2 d-blocks each)

BF16 = mybir.dt.bfloat16
FP32 = mybir.dt.float32
INT8 = mybir.dt.int8


def build_nc():
    nc = bacc.Bacc(None, target_bir_lowering=False)

    # host-pretransposed, tile-layout inputs (contiguous per-partition runs):
    # xt[q*128+p, db*T+t]   = x_shard[t, (q*XB+db)*128+p]
    # wqt[c*128+p, db*512+o] = w[db*128+p, c*512+o]  (d-major int8, all slices)
    xt_in = nc.dram_tensor("xt", [XQ * P, (D_BLKS // XQ) * T], BF16,
                           kind="ExternalInput")
    wqt_in = nc.dram_tensor("wqt", [O_CHUNKS * P, D_BLKS * O_CHUNK], INT8,
                            kind="ExternalInput")
    # scb[i][p, o] = bf16(s_i[o]) replicated across partitions
    scb_in = [
        nc.dram_tensor(f"scb{i}", [P, O], BF16, kind="ExternalInput")
        for i in range(N_SLICES)
    ]
    # biasc[p, G] = bias[G*128 + p] for global o-tile G
    biasc_in = nc.dram_tensor("biasc", [P, O // P], FP32, kind="ExternalInput")
    # output stored transposed in bf16 (host casts back; ~2e-3 extra rounding)
    outT = nc.dram_tensor("outT", [O, T], BF16, kind="ExternalOutput")

    with tile.TileContext(nc) as tc:
        with (
            tc.tile_pool(name="const", bufs=1) as const,
            tc.tile_pool(name="xres", bufs=1) as xres,
            tc.tile_pool(name="wt", bufs=2) as wt_pool,
            tc.tile_pool(name="ostage", bufs=2) as ostage,
            tc.tile_pool(name="psm", bufs=1, space="PSUM") as psm,
        ):
            # scales/bias ride the ACT HWDGE queue so the gpsimd queue is
            # free for the critical first weight chunk
            scbs = []
            for i in range(N_SLICES):
                sct = const.tile([P, O], BF16, tag=f"scb{i}")
                if i == 0:
                    # split so chunk 0's first dequant is gated by a small DMA
                    nc.scalar.dma_start(sct[:, 0:O_CHUNK],
                                        scb_in[i][:, 0:O_CHUNK])
                    nc.scalar.dma_start(sct[:, O_CHUNK:],
                                        scb_in[i][:, O_CHUNK:])
                else:
                    nc.scalar.dma_start(sct[:], scb_in[i][:])
                scbs.append(sct)
            biasc = const.tile([P, O // P], FP32)
            nc.scalar.dma_start(biasc[:], biasc_in[:])

            # x: [4096, 1024] bf16 -> XQ tiles [128, 32/XQ, 1024],
            # d = (q*(32/XQ) + db)*128 + p
            xTq = []
            for q in range(XQ):
                xq = xres.tile([P, D_BLKS // XQ, T], BF16, tag=f"xTq{q}",
                               name=f"xTq{q}")
                nc.sync.dma_start(
                    xq[:],
                    xt_in[q * P:(q + 1) * P, :]
                    .rearrange("p (db t) -> p db t", t=T),
                )
                xTq.append(xq)

            def load_wt_chunk(c, split_first=False):
                # weight chunk: [4096d, 512o] int8 -> [128, 32db, 512] bf16,
                # one cast-DMA + dequant per slice so slice 0 is ready early.
                # split_first halves slice 0's load/dequant (chunk 0 startup).
                wt = wt_pool.tile([P, D_BLKS, O_CHUNK], BF16, tag="wt",
                                  name="wt")
                for i in range(N_SLICES):
                    parts = 2 if (split_first and i == 0) else 1
                    nb = D_BLKS_SLICE // parts
                    for k in range(parts):
                        db0 = i * D_BLKS_SLICE + k * nb
                        sl = slice(db0, db0 + nb)
                        nc.gpsimd.dma_start(
                            wt[:, sl, :],
                            wqt_in[c * P:(c + 1) * P,
                                   db0 * O_CHUNK:(db0 + nb) * O_CHUNK]
                            .rearrange("p (db o) -> p db o", o=O_CHUNK),
                        )
                        nc.vector.tensor_tensor(
                            wt[:, sl, :], wt[:, sl, :],
                            scbs[i][:, None, c * O_CHUNK:(c + 1) * O_CHUNK]
                            .to_broadcast((P, nb, O_CHUNK)),
                            mybir.AluOpType.mult,
                        )
                return wt

            def drain(ob, ps, c, g, h):
                # PSUM -> SBUF bf16 with fp32 bias add on ACT
                G = c * O_TILES_PER_CHUNK + g
                nc.scalar.activation(
                    ob[:, g, h * T_HALF:(h + 1) * T_HALF],
                    ps[:],
                    mybir.ActivationFunctionType.Identity,
                    bias=biasc[:, G:G + 1],
                    scale=1.0,
                )

            def store_g(ob, c, g):
                G = c * O_TILES_PER_CHUNK + g
                nc.sync.dma_start(outT[G * P:(G + 1) * P, :], ob[:, g, :])

            def mk_ps(g, h):
                k = g * 2 + h
                return psm.tile([P, T_HALF], FP32, tag=f"ps{k}",
                                name=f"ps{k}")

            XB = D_BLKS // XQ      # d-blocks per x tile

            def rhs(db, h):
                return xTq[db // XB][:, db % XB,
                                     h * T_HALF:(h + 1) * T_HALF]

            # ---- chunk 0: db-outer over all 8 psum banks, so the PE
            # consumes x quarter tiles as they arrive ----
            wt = load_wt_chunk(0, split_first=True)
            ob = ostage.tile([P, O_TILES_PER_CHUNK, T], BF16, tag="ob",
                             name="ob")
            ps_all = [[mk_ps(g, h) for h in range(2)]
                      for g in range(O_TILES_PER_CHUNK)]
            for db in range(D_BLKS):
                for g in range(O_TILES_PER_CHUNK):
                    lhsT = wt[:, db, g * P:(g + 1) * P]
                    for h in range(2):
                        nc.tensor.matmul(
                            ps_all[g][h][:], lhsT, rhs(db, h),
                            start=(db == 0), stop=(db == D_BLKS - 1),
                        )
            for g in range(O_TILES_PER_CHUNK):
                for h in range(2):
                    drain(ob, ps_all[g][h], 0, g, h)
                store_g(ob, 0, g)

            # ---- chunks 1..7: g-outer (staggered drains) ----
            for c in range(1, O_CHUNKS):
                wt = load_wt_chunk(c)
                ob = ostage.tile([P, O_TILES_PER_CHUNK, T], BF16, tag="ob",
                                 name="ob")
                for g in range(O_TILES_PER_CHUNK):
                    ps = [mk_ps(g, h) for h in range(2)]
                    for db in range(D_BLKS):
                        lhsT = wt[:, db, g * P:(g + 1) * P]
                        for h in range(2):
                            nc.tensor.matmul(
                                ps[h][:], lhsT, rhs(db, h),
                                start=(db == 0), stop=(db == D_BLKS - 1),
                            )
                    for h in range(2):
                        drain(ob, ps[h], c, g, h)
                    if c == O_CHUNKS - 1 and g == O_TILES_PER_CHUNK - 1:
                        # overlap the final drains/stores on two HWDGE queues
                        G = c * O_TILES_PER_CHUNK + g
                        nc.sync.dma_start(
                            outT[G * P:(G + 1) * P, 0:T_HALF],
                            ob[:, g, 0:T_HALF])
                        nc.scalar.dma_start(
                            outT[G * P:(G + 1) * P, T_HALF:],
                            ob[:, g, T_HALF:])
                    else:
                        store_g(ob, c, g)
    nc.compile()
    return nc


_NC_CACHE = None


def _get_nc():
    global _NC_CACHE
    if _NC_CACHE is None:
        _NC_CACHE = build_nc()
    return _NC_CACHE


def _prep_inputs(x, wqs, ss, bias):
    xb = np.asarray(x, dtype=np.float32).reshape(B * S, D).astype(
        ml_dtypes.bfloat16)
    # wqT [4096d, 4096o] -> chunk-major tile layout
    # [8c, 128p, 32db, 512o] with d = db*128+p, o_full = c*512+o
    wqt_d = np.concatenate(
        [np.asarray(w).astype(np.int8).T for w in wqs], axis=0)
    wqt = np.ascontiguousarray(
        wqt_d.reshape(D_BLKS, P, O_CHUNKS, O_CHUNK)
        .transpose(2, 1, 0, 3)
        .reshape(O_CHUNKS * P, D_BLKS * O_CHUNK))
    scbs = [
        np.ascontiguousarray(
            np.broadcast_to(
                np.asarray(s, dtype=np.float32).astype(ml_dtypes.bfloat16),
                (P, O)))
        for s in ss
    ]
    biasc = np.ascontiguousarray(
        np.asarray(bias, dtype=np.float32).reshape(O // P, P).T)
    XB = D_BLKS // XQ
    in_maps = []
    for c in range(N_CORES):
        # x shard [T, D] -> [XQ*128, XB*T] with row q*128+p, col db*T+t
        xcore = (xb[c * T:(c + 1) * T, :].T
                 .reshape(XQ, XB, P, T)
                 .transpose(0, 2, 1, 3)
                 .reshape(XQ * P, XB * T))
        m = {
            "xt": np.ascontiguousarray(xcore),
            "wqt": wqt,
            "biasc": biasc,
        }
        for i in range(N_SLICES):
            m[f"scb{i}"] = scbs[i]
        in_maps.append(m)
    return in_maps


def run_on_hw(x, wqs, ss, bias, **spmd_kwargs):
    """Run and return (out_full [B,S,O] fp32, BassKernelResults)."""
    nc = _get_nc()
    in_maps = _prep_inputs(x, wqs, ss, bias)
    res = run_bass_kernel_spmd(nc, in_maps, core_ids=list(range(N_CORES)),
                               **spmd_kwargs)
    # each core returns outT [O, T]; tokens concatenate along axis 1
    out = np.concatenate([r["outT"] for r in res.results], axis=1)
    return np.ascontiguousarray(out.T.reshape(B, S, O).astype(np.float32)), res


def kernel(x, wq0, s0, wq1, s1, wq2, s2, wq3, s3, bias):
    out, _ = run_on_hw(x, [wq0, wq1, wq2, wq3], [s0, s1, s2, s3], bias)
    return out
